# revision 1
# baseline (speedup 1.0000x reference)
"""MoE router (top-2 gating) Trainium2 Bass kernel, SPMD over 8 NeuronCores.

Problem: x [4, 4096, 2048] f32, gate_w [64, 2048] f32.
  logits = x @ gate_w.T          -> [4, 4096, 64]
  scores, indices = top_k(logits, 2)
  weights = softmax(scores)      -> ([4, 4096, 2] f32, [4, 4096, 2] i32)

Sharding: data-parallel over tokens; each of the 8 cores gets 2048 tokens.
The per-core shard is shipped transposed ([D, T] layout) so the contraction
dim D lands on SBUF partitions and the PE streams tokens as the moving
operand of exact-fp32 matmuls (no on-device transposition of x needed).

Per-core device pipeline:
  - gate_w.T pre-laid-out [128, 16*64] (host) -> SBUF once
  - 2 chunks x 1024 tokens, each DMA'd as 4 sub-transfers of 2 MiB
    (4 d-tiles x 1024 tokens each) so matmuls start early
  - per d-tile, 4 fp32 matmuls (= 2 col-packed pairs, tile_position (0,0)
    and (0,64)) accumulate logits.T for four 256-token groups in 4 PSUM banks
  - per-chunk epilogue: PSUM->SBUF copies (DVE+ACT split), PE back-transpose
    to [token, expert] layout, hardware top-8 (max8 + max_index) for top-2
  - tail: batched softmax on top-2 scores; compacted contiguous output DMAs
  - host unpermutes the [128, 16, 2] per-core buffers to token order
"""
import sys

if "/opt/trn_rl_repo" not in sys.path:
    sys.path.insert(0, "/opt/trn_rl_repo")

import numpy as np

B, T, D, E, K = 4, 4096, 2048, 64, 2
N_CORES = 8
P = 128
NDT = D // P                      # 16 d-tiles
TOK_PER_CORE = B * T // N_CORES   # 2048
CHUNK = 1024                      # tokens per chunk
NCHUNK = TOK_PER_CORE // CHUNK    # 2
GRP = 256                         # tokens per packed col-group (matmul N)
NSEG = TOK_PER_CORE // P          # 16 output segments of 128 tokens

_compiled = None


def _build():
    import concourse.bacc as bacc
    import concourse.tile as tile
    from concourse import mybir
    from concourse.masks import make_identity

    nc = bacc.Bacc("TRN2", target_bir_lowering=False, debug=False,
                   num_devices=N_CORES)

    xT_in = nc.dram_tensor("xT", [D, TOK_PER_CORE], mybir.dt.float32,
                           kind="ExternalInput")
    gw_in = nc.dram_tensor("gwl", [P, NDT * E], mybir.dt.float32,
                           kind="ExternalInput")
    # single merged output: [:, 0:NSEG*K] = weight bits (f32), rest = indices
    o_out = nc.dram_tensor("o", [P, NSEG * K * 2], mybir.dt.uint32,
                           kind="ExternalOutput")

    fp32 = mybir.dt.float32
    # two 1024-token chunks, each streamed as four 2-MiB sub-DMAs on the
    # single sync HWDGE queue (measured fastest; finer splits or extra DMA
    # queues slow the whole stream down)
    CHUNKS = [(0, 1024), (1024, 1024)]
    SPLITS = {0: [(0, 4), (4, 4), (8, 4), (12, 4)],
              1024: [(0, 4), (4, 4), (8, 4), (12, 4)]}

    with tile.TileContext(nc) as tc:
        with (
            tc.tile_pool(name="xpool", bufs=1) as xpool,
            tc.tile_pool(name="cpool", bufs=1) as cpool,
            tc.tile_pool(name="epool", bufs=2) as epool,
            tc.tile_pool(name="opool", bufs=1) as opool,
            tc.tile_pool(name="pacc", bufs=2, space="PSUM") as pacc,
            tc.tile_pool(name="plg", bufs=2, space="PSUM") as plg,
        ):
            # constants / one-time loads
            gw_sb = cpool.tile([P, NDT * E], fp32)
            nc.sync.dma_start(gw_sb[:], gw_in.ap())
            ident = cpool.tile([P, P], fp32)
            make_identity(nc, ident[:])
            # warm the ACT exp table early (overlaps first DMA)
            scratch = cpool.tile([P, 1], fp32)
            nc.gpsimd.memset(scratch[:], 0.0)
            nc.scalar.activation(scratch[:], scratch[:],
                                 mybir.ActivationFunctionType.Exp)

            # per-core accumulators
            mx_acc = opool.tile([P, NSEG * 8], fp32)
            mi_acc = opool.tile([P, NSEG * 8], mybir.dt.uint32)
            acc_all = opool.tile([P, NSEG * K * 2], mybir.dt.uint32)
            mx3 = mx_acc[:].rearrange("p (s k) -> p s k", k=8)
            wv = acc_all[:, 0:NSEG * K].bitcast(fp32).rearrange(
                "p (s k) -> p s k", k=K)

            # xT DRAM view: [dt, p, t]
            xT_v = xT_in.ap().rearrange("(dt p) t -> p dt t", p=P)

            for ci, (t0, ntok) in enumerate(CHUNKS):
                half = ntok // 2          # tokens per packed col-group
                nblk = ntok // P          # back-transpose blocks
                # sub-DMAs; all tiles stay resident (unique tags)
                quarters = []
                for (d0, nd) in SPLITS[t0]:
                    xt = xpool.tile([P, nd * ntok], fp32,
                                    tag=f"x{ci}_{d0}", name=f"xt_{ci}_{d0}")
                    nc.sync.dma_start(
                        xt[:].rearrange("p (dt t) -> p dt t", dt=nd),
                        xT_v[:, d0:d0 + nd, t0:t0 + ntok],
                    )
                    quarters.append((d0, nd, xt))

                def find_src(dt):
                    for (d0, nd, xt) in quarters:
                        if d0 <= dt < d0 + nd:
                            return xt, (dt - d0) * ntok
                    raise AssertionError

                # one col-packed pair of fp32 matmuls per d-tile:
                #   group A = tokens [0, half)   -> cols 0:64  of the PE
                #   group B = tokens [half, ntok) -> cols 64:128
                pga = pacc.tile([64, half], fp32, tag="gA", name=f"pga{ci}")
                pgb = pacc.tile([P, half], fp32, tag="gB", name=f"pgb{ci}")
                for dt in range(NDT):
                    src, base = find_src(dt)
                    gsl = gw_sb[:, dt * E:(dt + 1) * E]
                    mmargs = dict(start=(dt == 0), stop=(dt == NDT - 1))
                    nc.tensor.matmul(pga[:], gsl, src[:, base:base + half],
                                     tile_position=(0, 0), **mmargs)
                    nc.tensor.matmul(pgb[64:128, :], gsl,
                                     src[:, base + half:base + ntok],
                                     tile_position=(0, 64), **mmargs)

                # epilogue: copy the two logits.T halves into their token-
                # aligned quadrants (other quadrants stay garbage, never read)
                lt = epool.tile([P, ntok], fp32, tag="lt", name=f"lt{ci}")
                nc.vector.tensor_copy(lt[0:64, 0:half], pga[:])
                nc.scalar.copy(lt[64:128, half:ntok], pgb[64:128, :])

                lg_ps = plg.tile([P, ntok], fp32, tag="lg_ps",
                                 name=f"lgps{ci}")
                for j in range(nblk):
                    nc.tensor.transpose(
                        lg_ps[:, j * P:(j + 1) * P], lt[:, j * P:(j + 1) * P],
                        ident[:],
                    )
                lg = epool.tile([P, ntok], fp32, tag="lg", name=f"lg{ci}")
                nc.vector.tensor_copy(lg[:, 0:ntok // 2], lg_ps[:, 0:ntok // 2])
                nc.scalar.copy(lg[:, ntok // 2:], lg_ps[:, ntok // 2:])

                # block j holds tokens t0+j*128+p; its valid 64 experts sit at
                # cols 0:64 (group A blocks) or 64:128 (group B blocks)
                for j in range(nblk):
                    s = t0 // P + j
                    cb = 0 if j < nblk // 2 else 64
                    seg = lg[:, j * P + cb: j * P + cb + 64]
                    nc.vector.max(out=mx_acc[:, s * 8:(s + 1) * 8], in_=seg)
                    nc.vector.max_index(
                        mi_acc[:, s * 8:(s + 1) * 8],
                        mx_acc[:, s * 8:(s + 1) * 8], seg,
                    )

                # per-chunk softmax + index compaction into the output tile
                s0, s1 = t0 // P, t0 // P + nblk
                nsg = s1 - s0
                delta = epool.tile([P, nsg], fp32, tag="delta",
                                   name=f"delta{ci}")
                nc.vector.tensor_tensor(delta[:], mx3[:, s0:s1, 1],
                                        mx3[:, s0:s1, 0],
                                        op=mybir.AluOpType.subtract)
                ee = epool.tile([P, nsg], fp32, tag="ee", name=f"ee{ci}")
                nc.scalar.activation(ee[:], delta[:],
                                     mybir.ActivationFunctionType.Exp)
                denom = epool.tile([P, nsg], fp32, tag="denom",
                                   name=f"denom{ci}")
                nc.vector.tensor_scalar_add(denom[:], ee[:], 1.0)
                nc.vector.reciprocal(wv[:, s0:s1, 0], denom[:])
                nc.vector.tensor_tensor(wv[:, s0:s1, 1], ee[:],
                                        wv[:, s0:s1, 0],
                                        op=mybir.AluOpType.mult)
                mi3 = mi_acc[:].rearrange("p (s k) -> p s k", k=8)
                nc.vector.tensor_copy(
                    acc_all[:, NSEG * K + s0 * K: NSEG * K + s1 * K]
                    .rearrange("p (s k) -> p s k", k=K),
                    mi3[:, s0:s1, 0:K])

                # ship this chunk's slice of both output halves now so only
                # the last chunk's 2x(nsg*K) columns ride the critical tail
                nc.sync.dma_start(o_out.ap()[:, s0 * K:s1 * K],
                                  acc_all[:, s0 * K:s1 * K])
                nc.sync.dma_start(
                    o_out.ap()[:, NSEG * K + s0 * K:NSEG * K + s1 * K],
                    acc_all[:, NSEG * K + s0 * K:NSEG * K + s1 * K])

    nc.compile()
    return nc


def _get_compiled():
    global _compiled
    if _compiled is None:
        _compiled = _build()
    return _compiled


def kernel(x, gate_w):
    from concourse.bass_utils import run_bass_kernel_spmd

    x = np.ascontiguousarray(np.asarray(x, dtype=np.float32))
    gate_w = np.ascontiguousarray(np.asarray(gate_w, dtype=np.float32))
    assert x.shape == (B, T, D) and gate_w.shape == (E, D)

    nc = _get_compiled()

    x_flat = x.reshape(B * T, D)
    # gate_w.T laid out [128, 16*64]: (p, dt*64+e) = gate_w[e, dt*128+p]
    gwl = np.ascontiguousarray(
        gate_w.T.reshape(NDT, P, E).transpose(1, 0, 2).reshape(P, NDT * E)
    )

    from concurrent.futures import ThreadPoolExecutor

    def shard(c):
        sl = x_flat[c * TOK_PER_CORE:(c + 1) * TOK_PER_CORE]
        return np.ascontiguousarray(sl.T)  # [D, TOK_PER_CORE]

    with ThreadPoolExecutor(max_workers=N_CORES) as ex:
        shards = list(ex.map(shard, range(N_CORES)))

    in_maps = [{"xT": shards[c], "gwl": gwl} for c in range(N_CORES)]
    res = run_bass_kernel_spmd(nc, in_maps, list(range(N_CORES)))

    # device buffer is [P, 2*NSEG*K] u32: first half f32 weight bits,
    # second half indices; token = s*128 + p
    def unperm(buf):
        return buf.reshape(P, NSEG, K).transpose(1, 0, 2).reshape(
            TOK_PER_CORE, K)

    ws, idxs = [], []
    for c in range(N_CORES):
        o = res.results[c]["o"]
        ws.append(unperm(o[:, :NSEG * K].view(np.float32)))
        idxs.append(unperm(o[:, NSEG * K:]))
    weights = np.concatenate(ws, axis=0).reshape(B, T, K).astype(np.float32)
    indices = np.concatenate(idxs, axis=0).reshape(B, T, K).astype(np.int32)
    return weights, indices



# revision 3
# speedup vs baseline: 1.0387x; 1.0387x over previous
"""MoE router (top-2 gating) Trainium2 Bass kernel, SPMD over 8 NeuronCores.

Problem: x [4, 4096, 2048] f32, gate_w [64, 2048] f32.
  logits = x @ gate_w.T          -> [4, 4096, 64]
  scores, indices = top_k(logits, 2)
  weights = softmax(scores)      -> ([4, 4096, 2] f32, [4, 4096, 2] i32)

Sharding: data-parallel over tokens; each of the 8 cores gets 2048 tokens,
shipped transposed ([D, T]) so the contraction dim D lands on SBUF
partitions and the PE streams tokens as the moving operand.

Schedule (v2, from trace analysis of the v1 kernel):
  - ALL input sub-DMAs are issued first on the sync HWDGE ring, in stream
    order; gw + output DMAs ride the scalar HWDGE ring so they can never
    stall the input stream (v1 lost ~8us: chunk-1 inputs sat in the sync
    FIFO behind chunk-0 output DMAs that wait on softmax).
  - chunks [1024, 512, 256, 256] tokens: big chunks while the stream is
    the bottleneck, small final chunk so the serial epilogue tail
    (transpose -> top-8 -> softmax -> out-DMA) after the last input byte
    is short.  Final chunk's d-tiles split into fine sub-DMAs so the PE
    chase after the last byte is ~1 d-tile.
  - PE warm-up matmuls at start (HAM clock-gate releases after ~4us of
    activity; v1 paid 2x-throttled matmuls after every DMA-wait gap).
  - softmax(top2) via two ACT sigmoids: w1 = sig(s1-s0), w0 = sig(-(s1-s0))
    instead of the 5-op exp/sum/recip chain.
"""
import sys

if "/opt/trn_rl_repo" not in sys.path:
    sys.path.insert(0, "/opt/trn_rl_repo")

import numpy as np

B, T, D, E, K = 4, 4096, 2048, 64, 2
N_CORES = 8
P = 128
NDT = D // P                      # 16 d-tiles
TOK_PER_CORE = B * T // N_CORES   # 2048
NSEG = TOK_PER_CORE // P          # 16 output segments of 128 tokens

# (token_start, ntok) per chunk; PSUM col-group packing needs ntok % 256 == 0
CHUNKS = [(0, 1024), (1024, 512), (1536, 256), (1792, 256)]
# d-tile sub-DMA splits per chunk (d0, ndt)
SPLITS = {
    0:    [(0, 4), (4, 4), (8, 4), (12, 4)],
    1024: [(0, 8), (8, 8)],
    1536: [(0, 16)],
    1792: [(0, 4), (4, 4), (8, 4), (12, 2), (14, 1), (15, 1)],
}
MODE = "fp32"          # "fp32" | "fp32r"
N_WARMUP = 24          # PE warm-up matmuls before the first real one

_compiled = None


def _build():
    import concourse.bacc as bacc
    import concourse.tile as tile
    from concourse import mybir
    from concourse.masks import make_identity

    nc = bacc.Bacc("TRN2", target_bir_lowering=False, debug=False,
                   num_devices=N_CORES)

    xT_in = nc.dram_tensor("xT", [D, TOK_PER_CORE], mybir.dt.float32,
                           kind="ExternalInput")
    gw_in = nc.dram_tensor("gwl", [P, NDT * E], mybir.dt.float32,
                           kind="ExternalInput")
    # single merged output: [:, 0:NSEG*K] = weight bits (f32), rest = indices
    o_out = nc.dram_tensor("o", [P, NSEG * K * 2], mybir.dt.uint32,
                           kind="ExternalOutput")

    fp32 = mybir.dt.float32
    mmdt = mybir.dt.float32r if MODE == "fp32r" else fp32

    def mdt(ap):
        return ap.bitcast(mmdt) if MODE == "fp32r" else ap

    with tile.TileContext(nc) as tc:
        with (
            tc.tile_pool(name="xpool", bufs=1) as xpool,
            tc.tile_pool(name="cpool", bufs=1) as cpool,
            tc.tile_pool(name="epool", bufs=2) as epool,
            tc.tile_pool(name="opool", bufs=1) as opool,
            tc.tile_pool(name="pacc", bufs=2, space="PSUM") as pacc,
            tc.tile_pool(name="plg", bufs=1, space="PSUM") as plg,
            tc.tile_pool(name="pwarm", bufs=1, space="PSUM") as pwarm,
        ):
            # ---- input stream: every sub-DMA issued first, sync ring ----
            xT_v = xT_in.ap().rearrange("(dt p) t -> p dt t", p=P)
            pieces = {}   # (t0, d0) -> (ndt, tile)
            for (t0, ntok) in CHUNKS:
                for (d0, nd) in SPLITS[t0]:
                    xt = xpool.tile([P, nd * ntok], fp32,
                                    tag=f"x{t0}_{d0}", name=f"xt_{t0}_{d0}")
                    nc.sync.dma_start(
                        xt[:].rearrange("p (dt t) -> p dt t", dt=nd),
                        xT_v[:, d0:d0 + nd, t0:t0 + ntok],
                    )
                    pieces[(t0, d0)] = (nd, xt)

            # ---- constants / one-time loads (scalar ring for gw) ----
            gw_sb = cpool.tile([P, NDT * E], fp32)
            nc.scalar.dma_start(gw_sb[:], gw_in.ap())
            ident = cpool.tile([P, P], fp32)
            make_identity(nc, ident[:])
            # warm the ACT sigmoid table early (overlaps input stream)
            scratch = cpool.tile([P, 1], fp32)
            nc.gpsimd.memset(scratch[:], 0.0)
            nc.scalar.activation(scratch[:], scratch[:],
                                 mybir.ActivationFunctionType.Sigmoid)

            # ---- PE warm-up: lift the HAM clock gate before real work ----
            warm = pwarm.tile([P, P], fp32, tag="warm", name="warm")
            for wi in range(N_WARMUP):
                nc.tensor.matmul(warm[:], ident[:], ident[:],
                                 start=True, stop=True)

            # ---- per-core accumulators ----
            mx_acc = opool.tile([P, NSEG * 8], fp32)
            mi_acc = opool.tile([P, NSEG * 8], mybir.dt.uint32)
            acc_all = opool.tile([P, NSEG * K * 2], mybir.dt.uint32)
            mx3 = mx_acc[:].rearrange("p (s k) -> p s k", k=8)
            wv = acc_all[:, 0:NSEG * K].bitcast(fp32).rearrange(
                "p (s k) -> p s k", k=K)
            mi3 = mi_acc[:].rearrange("p (s k) -> p s k", k=8)

            for ci, (t0, ntok) in enumerate(CHUNKS):
                half = ntok // 2          # tokens per packed col-group
                nblk = ntok // P          # back-transpose blocks

                def find_src(dt):
                    for (d0, nd, xt) in [(d0, *pieces[(t0, d0)])
                                         for (d0, _) in SPLITS[t0]]:
                        if d0 <= dt < d0 + nd:
                            return xt, (dt - d0) * ntok
                    raise AssertionError

                # one col-packed pair of matmuls per d-tile:
                #   group A = tokens [0, half)    -> PE cols 0:64
                #   group B = tokens [half, ntok) -> PE cols 64:128
                pga = pacc.tile([64, 512], fp32, tag="gA", name=f"pga{ci}")
                pgb = pacc.tile([P, 512], fp32, tag="gB", name=f"pgb{ci}")
                for dt in range(NDT):
                    src, base = find_src(dt)
                    gsl = mdt(gw_sb[:, dt * E:(dt + 1) * E])
                    mmargs = dict(start=(dt == 0), stop=(dt == NDT - 1))
                    nc.tensor.matmul(pga[:, :half], gsl,
                                     mdt(src[:, base:base + half]),
                                     tile_position=(0, 0), **mmargs)
                    nc.tensor.matmul(pgb[64:128, :half], gsl,
                                     mdt(src[:, base + half:base + ntok]),
                                     tile_position=(0, 64), **mmargs)

                # epilogue: copy the two logits.T halves into their token-
                # aligned quadrants (other quadrants stay garbage, never read)
                lt = epool.tile([P, 1024], fp32, tag="lt", name=f"lt{ci}")
                nc.vector.tensor_copy(lt[0:64, 0:half], pga[:, :half])
                nc.scalar.copy(lt[64:128, half:ntok], pgb[64:128, :half])

                lg_ps = plg.tile([P, 1024], fp32, tag="lg_ps",
                                 name=f"lgps{ci}")
                for j in range(nblk):
                    nc.tensor.transpose(
                        lg_ps[:, j * P:(j + 1) * P],
                        mdt(lt[:, j * P:(j + 1) * P]), mdt(ident[:]),
                    )
                lg = epool.tile([P, 1024], fp32, tag="lg", name=f"lg{ci}")
                hh = (nblk // 2) * P
                nc.vector.tensor_copy(lg[:, 0:hh], lg_ps[:, 0:hh])
                nc.scalar.copy(lg[:, hh:ntok], lg_ps[:, hh:ntok])

                # block j holds tokens t0+j*128+p; its valid 64 experts sit at
                # cols 0:64 (group A blocks) or 64:128 (group B blocks)
                for j in range(nblk):
                    s = t0 // P + j
                    cb = 0 if j < nblk // 2 else 64
                    seg = lg[:, j * P + cb: j * P + cb + 64]
                    nc.vector.max(out=mx_acc[:, s * 8:(s + 1) * 8], in_=seg)
                    nc.vector.max_index(
                        mi_acc[:, s * 8:(s + 1) * 8],
                        mx_acc[:, s * 8:(s + 1) * 8], seg,
                    )

                # top-2 softmax: w1 = sigmoid(s1-s0), w0 = sigmoid(s0-s1)
                s0, s1 = t0 // P, t0 // P + nblk
                nsg = s1 - s0
                delta = epool.tile([P, 16], fp32, tag="delta",
                                   name=f"delta{ci}")
                nc.vector.tensor_tensor(delta[:, :nsg], mx3[:, s0:s1, 1],
                                        mx3[:, s0:s1, 0],
                                        op=mybir.AluOpType.subtract)
                nc.scalar.activation(wv[:, s0:s1, 1], delta[:, :nsg],
                                     mybir.ActivationFunctionType.Sigmoid)
                nc.scalar.activation(wv[:, s0:s1, 0], delta[:, :nsg],
                                     mybir.ActivationFunctionType.Sigmoid,
                                     scale=-1.0)
                nc.vector.tensor_copy(
                    acc_all[:, NSEG * K + s0 * K: NSEG * K + s1 * K]
                    .rearrange("p (s k) -> p s k", k=K),
                    mi3[:, s0:s1, 0:K])

                # ship this chunk's outputs on the scalar ring (never blocks
                # the input stream)
                nc.scalar.dma_start(
                    o_out.ap()[:, NSEG * K + s0 * K:NSEG * K + s1 * K],
                    acc_all[:, NSEG * K + s0 * K:NSEG * K + s1 * K])
                nc.scalar.dma_start(o_out.ap()[:, s0 * K:s1 * K],
                                    acc_all[:, s0 * K:s1 * K])

    nc.compile()
    return nc


def _get_compiled():
    global _compiled
    if _compiled is None:
        _compiled = _build()
    return _compiled


def kernel(x, gate_w):
    from concourse.bass_utils import run_bass_kernel_spmd

    x = np.ascontiguousarray(np.asarray(x, dtype=np.float32))
    gate_w = np.ascontiguousarray(np.asarray(gate_w, dtype=np.float32))
    assert x.shape == (B, T, D) and gate_w.shape == (E, D)

    nc = _get_compiled()

    x_flat = x.reshape(B * T, D)
    # gate_w.T laid out [128, 16*64]: (p, dt*64+e) = gate_w[e, dt*128+p]
    gwl = np.ascontiguousarray(
        gate_w.T.reshape(NDT, P, E).transpose(1, 0, 2).reshape(P, NDT * E)
    )

    from concurrent.futures import ThreadPoolExecutor

    def shard(c):
        sl = x_flat[c * TOK_PER_CORE:(c + 1) * TOK_PER_CORE]
        return np.ascontiguousarray(sl.T)  # [D, TOK_PER_CORE]

    with ThreadPoolExecutor(max_workers=N_CORES) as ex:
        shards = list(ex.map(shard, range(N_CORES)))

    in_maps = [{"xT": shards[c], "gwl": gwl} for c in range(N_CORES)]
    res = run_bass_kernel_spmd(nc, in_maps, list(range(N_CORES)))

    # device buffer is [P, 2*NSEG*K] u32: first half f32 weight bits,
    # second half indices; token = s*128 + p
    def unperm(buf):
        return buf.reshape(P, NSEG, K).transpose(1, 0, 2).reshape(
            TOK_PER_CORE, K)

    ws, idxs = [], []
    for c in range(N_CORES):
        o = res.results[c]["o"]
        ws.append(unperm(o[:, :NSEG * K].view(np.float32)))
        idxs.append(unperm(o[:, NSEG * K:]))
    weights = np.concatenate(ws, axis=0).reshape(B, T, K).astype(np.float32)
    indices = np.concatenate(idxs, axis=0).reshape(B, T, K).astype(np.int32)
    return weights, indices


# revision 4
# speedup vs baseline: 1.0656x; 1.0259x over previous
"""MoE router (top-2 gating) Trainium2 Bass kernel, SPMD over 8 NeuronCores.

Problem: x [4, 4096, 2048] f32, gate_w [64, 2048] f32.
  logits = x @ gate_w.T          -> [4, 4096, 64]
  scores, indices = top_k(logits, 2)
  weights = softmax(scores)      -> ([4, 4096, 2] f32, [4, 4096, 2] i32)

Sharding: data-parallel over tokens; each of the 8 cores gets 2048 tokens,
shipped transposed ([D, T]) so the contraction dim D lands on SBUF
partitions and the PE streams tokens as the moving operand of exact-fp32
matmuls (min top2/top3 logit gap on this input is 4e-7 — any reduced
precision flips indices, so the matmul must stay fp32).

Schedule (v3, from trace analysis):
  - ALL 8 input sub-DMAs issued first on the sync HWDGE ring; gw + output
    DMAs ride the scalar HWDGE ring (v1 stalled the input stream ~8us
    behind an output DMA waiting on softmax).
  - Every input piece keeps a >=256-token extent so DRAM rows stay >=1KB
    (fine token splits exploded the descriptor count in v2 and made the
    stream issue-bound: 2048-desc pieces took 8-12us to generate).
  - Pieces: 4x(4dt x tok[0:1024]), 2x(6dt x tok[1024:2048]),
    (4dt x tok[1024:1792]), (4dt x tok[1792:2048]).  The last piece is
    0.5 MiB so only 256 tokens' worth of top-k trails the stream.
  - Compute chunks [1024, 768, 256] slice into those pieces; chunk 1+2
    accumulate interleaved per d-tile as B pieces land.
  - PE warm-up matmuls lift the HAM clock gate before real work.
  - softmax(top2) via two ACT sigmoids (w1 = sig(d), w0 = sig(-d)).
  - index-half output DMAs go on the (idle) sync ring as soon as indices
    are compacted; weight halves follow sigmoids on the scalar ring.
"""
import sys

if "/opt/trn_rl_repo" not in sys.path:
    sys.path.insert(0, "/opt/trn_rl_repo")

import numpy as np

B, T, D, E, K = 4, 4096, 2048, 64, 2
N_CORES = 8
P = 128
NDT = D // P                      # 16 d-tiles
TOK_PER_CORE = B * T // N_CORES   # 2048
NSEG = TOK_PER_CORE // P          # 16 output segments of 128 tokens

# input pieces: (name, d0, ndt, t0, ntok), issued in this order
PIECES = [
    ("a0", 0, 4, 0, 1024),
    ("a1", 4, 4, 0, 1024),
    ("a2", 8, 4, 0, 1024),
    ("a3", 12, 4, 0, 1024),
    ("b0", 0, 6, 1024, 1024),
    ("b1", 6, 6, 1024, 1024),
    ("b2", 12, 4, 1024, 768),
    ("b3", 12, 4, 1792, 256),
]
# compute chunks: (t0, ntok)
CHUNKS = [(0, 1024), (1024, 768), (1792, 256)]
N_WARMUP = 24          # PE warm-up matmuls before the first real one

_compiled = None


def _build():
    import concourse.bacc as bacc
    import concourse.tile as tile
    from concourse import mybir
    from concourse.masks import make_identity

    nc = bacc.Bacc("TRN2", target_bir_lowering=False, debug=False,
                   num_devices=N_CORES)

    xT_in = nc.dram_tensor("xT", [D, TOK_PER_CORE], mybir.dt.float32,
                           kind="ExternalInput")
    gw_in = nc.dram_tensor("gwl", [P, NDT * E], mybir.dt.float32,
                           kind="ExternalInput")
    # single merged output: [:, 0:NSEG*K] = weight bits (f32), rest = indices
    o_out = nc.dram_tensor("o", [P, NSEG * K * 2], mybir.dt.uint32,
                           kind="ExternalOutput")

    fp32 = mybir.dt.float32

    with tile.TileContext(nc) as tc:
        with (
            tc.tile_pool(name="xpool", bufs=1) as xpool,
            tc.tile_pool(name="cpool", bufs=1) as cpool,
            tc.tile_pool(name="epool", bufs=2) as epool,
            tc.tile_pool(name="opool", bufs=1) as opool,
            tc.tile_pool(name="pacc", bufs=2, space="PSUM") as pacc,
            tc.tile_pool(name="plg", bufs=2, space="PSUM") as plg,
            tc.tile_pool(name="pwarm", bufs=1, space="PSUM") as pwarm,
        ):
            # ---- input stream: every sub-DMA issued first, sync ring ----
            xT_v = xT_in.ap().rearrange("(dt p) t -> p dt t", p=P)
            pieces = {}   # name -> (d0, ndt, t0, ntok, tile)
            for (nm, d0, nd, t0, ntok) in PIECES:
                xt = xpool.tile([P, nd * ntok], fp32,
                                tag=f"x_{nm}", name=f"xt_{nm}")
                nc.sync.dma_start(
                    xt[:].rearrange("p (dt t) -> p dt t", dt=nd),
                    xT_v[:, d0:d0 + nd, t0:t0 + ntok],
                )
                pieces[nm] = (d0, nd, t0, ntok, xt)

            def src_ap(dt, ct0, cn):
                """moving operand slice for d-tile dt, tokens [ct0, ct0+cn)"""
                for (d0, nd, t0, ntok, xt) in pieces.values():
                    if d0 <= dt < d0 + nd and t0 <= ct0 and ct0 + cn <= t0 + ntok:
                        base = (dt - d0) * ntok + (ct0 - t0)
                        return xt[:, base:base + cn]
                raise AssertionError((dt, ct0, cn))

            # ---- constants / one-time loads (scalar ring for gw) ----
            gw_sb = cpool.tile([P, NDT * E], fp32)
            nc.scalar.dma_start(gw_sb[:], gw_in.ap())
            ident = cpool.tile([P, P], fp32)
            make_identity(nc, ident[:])
            # warm the ACT sigmoid table early (overlaps input stream)
            scratch = cpool.tile([P, 1], fp32)
            nc.gpsimd.memset(scratch[:], 0.0)
            nc.scalar.activation(scratch[:], scratch[:],
                                 mybir.ActivationFunctionType.Sigmoid)

            # ---- PE warm-up: lift the HAM clock gate before real work ----
            warm = pwarm.tile([P, P], fp32, tag="warm", name="warm")
            for wi in range(N_WARMUP):
                nc.tensor.matmul(warm[:], ident[:], ident[:],
                                 start=True, stop=True)

            # ---- per-core accumulators ----
            mx_acc = opool.tile([P, NSEG * 8], fp32)
            mi_acc = opool.tile([P, NSEG * 8], mybir.dt.uint32)
            acc_all = opool.tile([P, NSEG * K * 2], mybir.dt.uint32)
            mx3 = mx_acc[:].rearrange("p (s k) -> p s k", k=8)
            wv = acc_all[:, 0:NSEG * K].bitcast(fp32).rearrange(
                "p (s k) -> p s k", k=K)
            mi3 = mi_acc[:].rearrange("p (s k) -> p s k", k=8)

            def emit_mm(ci, ct0, cn, pga, pgb, dts):
                half = cn // 2
                for dt in dts:
                    gsl = gw_sb[:, dt * E:(dt + 1) * E]
                    mmargs = dict(start=(dt == 0), stop=(dt == NDT - 1))
                    nc.tensor.matmul(pga[:, :half], gsl,
                                     src_ap(dt, ct0, half),
                                     tile_position=(0, 0), **mmargs)
                    nc.tensor.matmul(pgb[64:128, :half], gsl,
                                     src_ap(dt, ct0 + half, half),
                                     tile_position=(0, 64), **mmargs)

            def emit_epilogue(ci, ct0, cn, pga, pgb):
                half = cn // 2
                nblk = cn // P
                s0 = ct0 // P
                # copy the two logits.T halves into token-aligned quadrants
                lt = epool.tile([P, 1024], fp32, tag="lt", name=f"lt{ci}")
                nc.vector.tensor_copy(lt[0:64, 0:half], pga[:, :half])
                nc.scalar.copy(lt[64:128, half:cn], pgb[64:128, :half])
                # back-transpose in passes of <=4 blocks (plg bank = 512 f32)
                lg = epool.tile([P, 1024], fp32, tag="lg", name=f"lg{ci}")
                for pi in range(0, nblk, 4):
                    pe = min(pi + 4, nblk)
                    lg_ps = plg.tile([P, 512], fp32, tag="lg_ps",
                                     name=f"lgps{ci}_{pi}")
                    for j in range(pi, pe):
                        nc.tensor.transpose(
                            lg_ps[:, (j - pi) * P:(j - pi + 1) * P],
                            lt[:, j * P:(j + 1) * P], ident[:],
                        )
                    cp = nc.vector.tensor_copy if (pi // 4) % 2 == 0 \
                        else nc.scalar.copy
                    cp(lg[:, pi * P:pe * P], lg_ps[:, 0:(pe - pi) * P])
                # per 128-token block: hardware top-8 then top-2 compaction
                for j in range(nblk):
                    s = s0 + j
                    cb = 0 if j < nblk // 2 else 64
                    seg = lg[:, j * P + cb: j * P + cb + 64]
                    nc.vector.max(out=mx_acc[:, s * 8:(s + 1) * 8], in_=seg)
                    nc.vector.max_index(
                        mi_acc[:, s * 8:(s + 1) * 8],
                        mx_acc[:, s * 8:(s + 1) * 8], seg,
                    )
                s1 = s0 + nblk
                nsg = nblk
                # indices out first (sync ring is idle once inputs issued)
                nc.vector.tensor_copy(
                    acc_all[:, NSEG * K + s0 * K: NSEG * K + s1 * K]
                    .rearrange("p (s k) -> p s k", k=K),
                    mi3[:, s0:s1, 0:K])
                nc.sync.dma_start(
                    o_out.ap()[:, NSEG * K + s0 * K:NSEG * K + s1 * K],
                    acc_all[:, NSEG * K + s0 * K:NSEG * K + s1 * K])
                # top-2 softmax: w1 = sigmoid(s1-s0), w0 = sigmoid(s0-s1)
                delta = epool.tile([P, 16], fp32, tag="delta",
                                   name=f"delta{ci}")
                nc.vector.tensor_tensor(delta[:, :nsg], mx3[:, s0:s1, 1],
                                        mx3[:, s0:s1, 0],
                                        op=mybir.AluOpType.subtract)
                nc.scalar.activation(wv[:, s0:s1, 1], delta[:, :nsg],
                                     mybir.ActivationFunctionType.Sigmoid)
                nc.scalar.activation(wv[:, s0:s1, 0], delta[:, :nsg],
                                     mybir.ActivationFunctionType.Sigmoid,
                                     scale=-1.0)
                nc.scalar.dma_start(o_out.ap()[:, s0 * K:s1 * K],
                                    acc_all[:, s0 * K:s1 * K])

            # chunk 0: tokens 0:1024 from the A pieces
            pga0 = pacc.tile([64, 512], fp32, tag="gA", name="pga0")
            pgb0 = pacc.tile([P, 512], fp32, tag="gB", name="pgb0")
            emit_mm(0, 0, 1024, pga0, pgb0, range(NDT))
            emit_epilogue(0, 0, 1024, pga0, pgb0)

            # chunks 1+2 accumulate interleaved as the B pieces land
            pga1 = pacc.tile([64, 512], fp32, tag="gA", name="pga1")
            pgb1 = pacc.tile([P, 512], fp32, tag="gB", name="pgb1")
            pga2 = pacc.tile([64, 512], fp32, tag="gA", name="pga2")
            pgb2 = pacc.tile([P, 512], fp32, tag="gB", name="pgb2")
            for dt in range(12):
                emit_mm(1, 1024, 768, pga1, pgb1, [dt])
                emit_mm(2, 1792, 256, pga2, pgb2, [dt])
            emit_mm(1, 1024, 768, pga1, pgb1, range(12, 16))
            emit_epilogue(1, 1024, 768, pga1, pgb1)
            emit_mm(2, 1792, 256, pga2, pgb2, range(12, 16))
            emit_epilogue(2, 1792, 256, pga2, pgb2)

    nc.compile()
    return nc


def _get_compiled():
    global _compiled
    if _compiled is None:
        _compiled = _build()
    return _compiled


def kernel(x, gate_w):
    from concourse.bass_utils import run_bass_kernel_spmd

    x = np.ascontiguousarray(np.asarray(x, dtype=np.float32))
    gate_w = np.ascontiguousarray(np.asarray(gate_w, dtype=np.float32))
    assert x.shape == (B, T, D) and gate_w.shape == (E, D)

    nc = _get_compiled()

    x_flat = x.reshape(B * T, D)
    # gate_w.T laid out [128, 16*64]: (p, dt*64+e) = gate_w[e, dt*128+p]
    gwl = np.ascontiguousarray(
        gate_w.T.reshape(NDT, P, E).transpose(1, 0, 2).reshape(P, NDT * E)
    )

    from concurrent.futures import ThreadPoolExecutor

    def shard(c):
        sl = x_flat[c * TOK_PER_CORE:(c + 1) * TOK_PER_CORE]
        return np.ascontiguousarray(sl.T)  # [D, TOK_PER_CORE]

    with ThreadPoolExecutor(max_workers=N_CORES) as ex:
        shards = list(ex.map(shard, range(N_CORES)))

    in_maps = [{"xT": shards[c], "gwl": gwl} for c in range(N_CORES)]
    res = run_bass_kernel_spmd(nc, in_maps, list(range(N_CORES)))

    # device buffer is [P, 2*NSEG*K] u32: first half f32 weight bits,
    # second half indices; token = s*128 + p
    def unperm(buf):
        return buf.reshape(P, NSEG, K).transpose(1, 0, 2).reshape(
            TOK_PER_CORE, K)

    ws, idxs = [], []
    for c in range(N_CORES):
        o = res.results[c]["o"]
        ws.append(unperm(o[:, :NSEG * K].view(np.float32)))
        idxs.append(unperm(o[:, NSEG * K:]))
    weights = np.concatenate(ws, axis=0).reshape(B, T, K).astype(np.float32)
    indices = np.concatenate(idxs, axis=0).reshape(B, T, K).astype(np.int32)
    return weights, indices


# revision 9
# speedup vs baseline: 1.1431x; 1.0727x over previous
"""MoE router (top-2 gating) Trainium2 Bass kernel, SPMD over 8 NeuronCores.

Problem: x [4, 4096, 2048] f32, gate_w [64, 2048] f32.
  logits = x @ gate_w.T          -> [4, 4096, 64]
  scores, indices = top_k(logits, 2)
  weights = softmax(scores)      -> ([4, 4096, 2] f32, [4, 4096, 2] i32)

Sharding: data-parallel over tokens; each of the 8 cores gets 2048 tokens,
shipped transposed ([D, T]) so the contraction dim D lands on SBUF
partitions and the PE streams tokens as the moving operand of exact-fp32
matmuls (min top2/top3 logit gap on this input is 4e-7 — any reduced
precision flips indices, so the matmul must stay fp32).

Schedule (v3, from trace analysis):
  - ALL 8 input sub-DMAs issued first on the sync HWDGE ring; gw + output
    DMAs ride the scalar HWDGE ring (v1 stalled the input stream ~8us
    behind an output DMA waiting on softmax).
  - Every input piece keeps a >=256-token extent so DRAM rows stay >=1KB
    (fine token splits exploded the descriptor count in v2 and made the
    stream issue-bound: 2048-desc pieces took 8-12us to generate).
  - Pieces: 4x(4dt x tok[0:1024]), 2x(6dt x tok[1024:2048]),
    (4dt x tok[1024:1792]), (4dt x tok[1792:2048]).  The last piece is
    0.5 MiB so only 256 tokens' worth of top-k trails the stream.
  - Compute chunks [1024, 768, 256] slice into those pieces; chunk 1+2
    accumulate interleaved per d-tile as B pieces land.
  - PE warm-up matmuls lift the HAM clock gate before real work.
  - softmax(top2) via two ACT sigmoids (w1 = sig(d), w0 = sig(-d)).
  - index-half output DMAs go on the (idle) sync ring as soon as indices
    are compacted; weight halves follow sigmoids on the scalar ring.
"""
import sys

if "/opt/trn_rl_repo" not in sys.path:
    sys.path.insert(0, "/opt/trn_rl_repo")

import numpy as np

B, T, D, E, K = 4, 4096, 2048, 64, 2
N_CORES = 8
P = 128
NDT = D // P                      # 16 d-tiles
TOK_PER_CORE = B * T // N_CORES   # 2048
NSEG = TOK_PER_CORE // P          # 16 output segments of 128 tokens

# input pieces: (name, d0, ndt, t0, ntok), issued in this order
PIECES = [
    ("a0", 0, 4, 0, 1024),
    ("a1", 4, 4, 0, 1024),
    ("a2", 8, 4, 0, 1024),
    ("a3", 12, 4, 0, 1024),
    ("b0", 0, 6, 1024, 1024),
    ("b1", 6, 6, 1024, 1024),
    ("b2", 12, 4, 1024, 512),
    ("b3", 12, 4, 1536, 512),
]
# compute chunks: (t0, ntok); chunk k completes when its last piece lands,
# staggered so each epilogue overlaps the remaining stream
CHUNKS = [(0, 1024), (1024, 512), (1536, 512)]
N_WARMUP = 24          # PE warm-up matmuls before the first real one

_compiled = None


def _build():
    import concourse.bacc as bacc
    import concourse.tile as tile
    from concourse import mybir
    from concourse.masks import make_identity

    nc = bacc.Bacc("TRN2", target_bir_lowering=False, debug=False,
                   num_devices=N_CORES)

    xT_in = nc.dram_tensor("xT", [D, TOK_PER_CORE], mybir.dt.float32,
                           kind="ExternalInput")
    gw_in = nc.dram_tensor("gwl", [P, NDT * E], mybir.dt.float32,
                           kind="ExternalInput")
    # single merged output: [:, 0:NSEG*K] = weight bits (f32), rest = indices
    o_out = nc.dram_tensor("o", [P, NSEG * K * 2], mybir.dt.uint32,
                           kind="ExternalOutput")

    fp32 = mybir.dt.float32

    with tile.TileContext(nc) as tc:
        with (
            tc.tile_pool(name="xpool", bufs=1) as xpool,
            tc.tile_pool(name="cpool", bufs=1) as cpool,
            tc.tile_pool(name="epool", bufs=2) as epool,
            tc.tile_pool(name="opool", bufs=1) as opool,
            tc.tile_pool(name="pacc", bufs=2, space="PSUM") as pacc,
            tc.tile_pool(name="plg", bufs=2, space="PSUM") as plg,
            tc.tile_pool(name="pwarm", bufs=1, space="PSUM") as pwarm,
        ):
            # ---- input stream: every sub-DMA issued first, sync ring ----
            xT_v = xT_in.ap().rearrange("(dt p) t -> p dt t", p=P)
            pieces = {}   # name -> (d0, ndt, t0, ntok, tile)
            for (nm, d0, nd, t0, ntok) in PIECES:
                xt = xpool.tile([P, nd * ntok], fp32,
                                tag=f"x_{nm}", name=f"xt_{nm}")
                nc.sync.dma_start(
                    xt[:].rearrange("p (dt t) -> p dt t", dt=nd),
                    xT_v[:, d0:d0 + nd, t0:t0 + ntok],
                )
                pieces[nm] = (d0, nd, t0, ntok, xt)

            def src_ap(dt, ct0, cn):
                """moving operand slice for d-tile dt, tokens [ct0, ct0+cn)"""
                for (d0, nd, t0, ntok, xt) in pieces.values():
                    if d0 <= dt < d0 + nd and t0 <= ct0 and ct0 + cn <= t0 + ntok:
                        base = (dt - d0) * ntok + (ct0 - t0)
                        return xt[:, base:base + cn]
                raise AssertionError((dt, ct0, cn))

            # ---- constants / one-time loads (scalar ring for gw) ----
            gw_sb = cpool.tile([P, NDT * E], fp32)
            nc.scalar.dma_start(gw_sb[:], gw_in.ap())
            ident = cpool.tile([P, P], fp32)
            make_identity(nc, ident[:])
            # warm the ACT sigmoid table early (overlaps input stream)
            scratch = cpool.tile([P, 1], fp32)
            nc.gpsimd.memset(scratch[:], 0.0)
            nc.scalar.activation(scratch[:], scratch[:],
                                 mybir.ActivationFunctionType.Sigmoid)

            # ---- PE warm-up: lift the HAM clock gate before real work ----
            warm = pwarm.tile([P, P], fp32, tag="warm", name="warm")
            for wi in range(N_WARMUP):
                nc.tensor.matmul(warm[:], ident[:], ident[:],
                                 start=True, stop=True)

            # ---- per-core accumulators ----
            mx_acc = opool.tile([P, NSEG * 8], fp32)
            mi_acc = opool.tile([P, NSEG * 8], mybir.dt.uint32)
            acc_all = opool.tile([P, NSEG * K * 2], mybir.dt.uint32)
            mx3 = mx_acc[:].rearrange("p (s k) -> p s k", k=8)
            wv = acc_all[:, 0:NSEG * K].bitcast(fp32).rearrange(
                "p (s k) -> p s k", k=K)
            mi3 = mi_acc[:].rearrange("p (s k) -> p s k", k=8)

            def emit_mm(ci, ct0, cn, pga, pgb, dts):
                half = cn // 2
                for dt in dts:
                    gsl = gw_sb[:, dt * E:(dt + 1) * E]
                    mmargs = dict(start=(dt == 0), stop=(dt == NDT - 1))
                    nc.tensor.matmul(pga[:, :half], gsl,
                                     src_ap(dt, ct0, half),
                                     tile_position=(0, 0), **mmargs)
                    nc.tensor.matmul(pgb[64:128, :half], gsl,
                                     src_ap(dt, ct0 + half, half),
                                     tile_position=(0, 64), **mmargs)

            def emit_epilogue(ci, ct0, cn, pga, pgb):
                half = cn // 2
                nblk = cn // P
                s0 = ct0 // P
                # copy the two logits.T halves into token-aligned quadrants
                # (gpsimd can't read PSUM, so DVE + scalar)
                lt = epool.tile([P, 1024], fp32, tag="lt", name=f"lt{ci}")
                nc.vector.tensor_copy(lt[0:64, 0:half], pga[:, :half])
                nc.scalar.copy(lt[64:128, half:cn], pgb[64:128, :half])
                # back-transpose in passes of <=4 blocks (plg bank = 512 f32);
                # top-8 / top-8-index read the transposed PSUM directly
                segs = {}
                for pi in range(0, nblk, 4):
                    pe = min(pi + 4, nblk)
                    last = pe == nblk
                    lg_ps = plg.tile([P, 512], fp32, tag="lg_ps",
                                     name=f"lgps{ci}_{pi}")
                    for j in range(pi, pe):
                        nc.tensor.transpose(
                            lg_ps[:, (j - pi) * P:(j - pi + 1) * P],
                            lt[:, j * P:(j + 1) * P], ident[:],
                        )
                    for j in range(pi, pe):
                        s = s0 + j
                        cb = 0 if j < nblk // 2 else 64
                        seg = lg_ps[:, (j - pi) * P + cb:
                                    (j - pi) * P + cb + 64]
                        segs[s] = seg
                        nc.vector.max(out=mx_acc[:, s * 8:(s + 1) * 8],
                                      in_=seg)
                        if not last:
                            nc.vector.max_index(
                                mi_acc[:, s * 8:(s + 1) * 8],
                                mx_acc[:, s * 8:(s + 1) * 8], seg,
                            )
                s1 = s0 + nblk
                nsg = nblk
                # weights path first: it only needs the max VALUES, so the
                # sigmoids + weight DMA (scalar ring) overlap the index
                # chain still running on the DVE
                delta = epool.tile([P, 16], fp32, tag="delta",
                                   name=f"delta{ci}")
                nc.vector.tensor_tensor(delta[:, :nsg], mx3[:, s0:s1, 1],
                                        mx3[:, s0:s1, 0],
                                        op=mybir.AluOpType.subtract)
                nc.scalar.activation(wv[:, s0:s1, 1], delta[:, :nsg],
                                     mybir.ActivationFunctionType.Sigmoid)
                nc.scalar.activation(wv[:, s0:s1, 0], delta[:, :nsg],
                                     mybir.ActivationFunctionType.Sigmoid,
                                     scale=-1.0)
                nc.scalar.dma_start(o_out.ap()[:, s0 * K:s1 * K],
                                    acc_all[:, s0 * K:s1 * K])
                # index chain for the final pass, then indices out (sync ring)
                lp = (nblk - 1) // 4 * 4
                for j in range(lp, nblk):
                    s = s0 + j
                    nc.vector.max_index(
                        mi_acc[:, s * 8:(s + 1) * 8],
                        mx_acc[:, s * 8:(s + 1) * 8], segs[s],
                    )
                nc.gpsimd.tensor_copy(
                    acc_all[:, NSEG * K + s0 * K: NSEG * K + s1 * K]
                    .rearrange("p (s k) -> p s k", k=K),
                    mi3[:, s0:s1, 0:K])
                nc.sync.dma_start(
                    o_out.ap()[:, NSEG * K + s0 * K:NSEG * K + s1 * K],
                    acc_all[:, NSEG * K + s0 * K:NSEG * K + s1 * K])

            # chunk 0: tokens 0:1024 from the A pieces
            pga0 = pacc.tile([64, 512], fp32, tag="gA", name="pga0")
            pgb0 = pacc.tile([P, 512], fp32, tag="gB", name="pgb0")
            emit_mm(0, 0, 1024, pga0, pgb0, range(NDT))
            emit_epilogue(0, 0, 1024, pga0, pgb0)

            # chunks 1+2 accumulate interleaved as the B pieces land
            pga1 = pacc.tile([64, 512], fp32, tag="gA", name="pga1")
            pgb1 = pacc.tile([P, 512], fp32, tag="gB", name="pgb1")
            pga2 = pacc.tile([64, 512], fp32, tag="gA", name="pga2")
            pgb2 = pacc.tile([P, 512], fp32, tag="gB", name="pgb2")
            for dt in range(12):
                emit_mm(1, 1024, 512, pga1, pgb1, [dt])
                emit_mm(2, 1536, 512, pga2, pgb2, [dt])
            emit_mm(1, 1024, 512, pga1, pgb1, range(12, 16))
            emit_epilogue(1, 1024, 512, pga1, pgb1)
            emit_mm(2, 1536, 512, pga2, pgb2, range(12, 16))
            emit_epilogue(2, 1536, 512, pga2, pgb2)

    nc.compile()
    return nc


def _get_compiled():
    global _compiled
    if _compiled is None:
        _compiled = _build()
    return _compiled


def kernel(x, gate_w):
    from concourse.bass_utils import run_bass_kernel_spmd

    x = np.ascontiguousarray(np.asarray(x, dtype=np.float32))
    gate_w = np.ascontiguousarray(np.asarray(gate_w, dtype=np.float32))
    assert x.shape == (B, T, D) and gate_w.shape == (E, D)

    nc = _get_compiled()

    x_flat = x.reshape(B * T, D)
    # gate_w.T laid out [128, 16*64]: (p, dt*64+e) = gate_w[e, dt*128+p]
    gwl = np.ascontiguousarray(
        gate_w.T.reshape(NDT, P, E).transpose(1, 0, 2).reshape(P, NDT * E)
    )

    from concurrent.futures import ThreadPoolExecutor

    def shard(c):
        sl = x_flat[c * TOK_PER_CORE:(c + 1) * TOK_PER_CORE]
        return np.ascontiguousarray(sl.T)  # [D, TOK_PER_CORE]

    with ThreadPoolExecutor(max_workers=N_CORES) as ex:
        shards = list(ex.map(shard, range(N_CORES)))

    in_maps = [{"xT": shards[c], "gwl": gwl} for c in range(N_CORES)]
    res = run_bass_kernel_spmd(nc, in_maps, list(range(N_CORES)))

    # device buffer is [P, 2*NSEG*K] u32: first half f32 weight bits,
    # second half indices; token = s*128 + p
    def unperm(buf):
        return buf.reshape(P, NSEG, K).transpose(1, 0, 2).reshape(
            TOK_PER_CORE, K)

    ws, idxs = [], []
    for c in range(N_CORES):
        o = res.results[c]["o"]
        ws.append(unperm(o[:, :NSEG * K].view(np.float32)))
        idxs.append(unperm(o[:, NSEG * K:]))
    weights = np.concatenate(ws, axis=0).reshape(B, T, K).astype(np.float32)
    indices = np.concatenate(idxs, axis=0).reshape(B, T, K).astype(np.int32)
    return weights, indices
